# revision 29
# baseline (speedup 1.0000x reference)
# Trainium2 Bass kernel for nn_CalculateAttention_7722351198463
#
# reference computes, per (batch, head):
#   scores = (Qx @ Kx^T + Qy @ Ky^T) * 0.5 / sqrt(D)
#   attn   = softmax(scores, axis=-1)
#   out1   = attn @ Vx ; out2 = attn @ Vy
#
# Sharding: B*H = 64 heads, 8 heads per core across 8 NeuronCores (no comms).
#
# Device-side design (per core, 8 heads), v2 (87.4us baseline -> 82.8us):
#  * Host pre-transposes Q/K into QT/KT = [d=128, s=1024] per head (x stream
#    on partitions 0:64, y on 64:128); one bf16 matmul with contraction 128
#    computes Qx@Kx^T + Qy@Ky^T in one pass, in transposed [t, s] layout.
#  * The core's QK output is a single global stream of 128 "slots" (one slot
#    = one N=512 matmul = one PSUM bank).  Slots ping-pong between TWO
#    3-bank PSUM tiles sA/sB; ScalarE exp() consumes a whole tile per
#    ACTIVATE (N=1536), cutting the ~260ns per-call ACT overhead 3x vs
#    per-1024 calls.  The exp stream is the kernel bottleneck (ScalarE =
#    128 lanes @ 1.2GHz: 54.6us + 44 x 0.26us overhead ~= 66us busy) and
#    runs gapless in steady state.  sA/sB MUST be separate tiles: PSUM dep
#    tracking conflates slices within one tile, which serialized QK behind
#    the previous ACTIVATE (+950ns/call).
#  * exp lands in one persistent write-once SBUF tensor EX[128, 128*512]
#    (bf16), so ACTIVATE calls may cross head boundaries.
#  * QK slots are c-major within a head (all 8 key-tiles for query columns
#    0:512, then 512:1024), so each PV group's accumulation spans ~2.7 ACT
#    calls and at most ~4 of 8 groups per head are PSUM-resident, fitting 2
#    PV banks (3 slices/bank at col offsets 0/130/260).  Slices alternate
#    banks group-to-group so evacuation casts (DVE) never share a bank with
#    live PV matmuls (same-bank PE-W + DVE-R is fatal -> Tile serializes).
#  * PV work is emitted as closures drained ~13 matmuls per ACT call with a
#    6-slot lag, so the PE queue alternates [3 QK][~13 PV]; a dep-blocked
#    PV matmul can never starve ACT of its next QK slots (strict PE FIFO).
#  * V is packed host-side as VC = [t, 132] = [Vx | Vy | ones | pad]: one
#    accumulating matmul chain per (head, s-block) computes
#    [out1_raw | out2_raw | sumexp]; normalization happens on host.
#  * Outputs are stored bf16 (raw + sumexp), halving output DMA.
#  * Ramp: exp-table load triggered immediately via a zero-dep warm
#    activation; head 0's DMA is split per-matmul so the first (mini)
#    ACTIVATE waits only on qt[:,0:512] + kt tile 0; later prefetch is
#    chained serially off the ramp-critical path.
#  * Tail: the final half runs j-outer across 4 distinct banks (pva, pvb +
#    2 idle sB banks) so only its j6/j7 waves trail the last ACTIVATE.
import numpy as np
import ml_dtypes

B, H, S, D = 4, 16, 1024, 64
N_CORES = 8
HEADS = B * H              # 64
HPC = HEADS // N_CORES     # heads per core = 8
ST = S // 128              # key tiles per head = 8
SCALE = 0.5 / 8.0          # 0.5 / sqrt(D)
VCW = 132                  # packed V width: 64 + 64 + 1 (ones) + 3 pad
INW = 2 * S + ST * VCW     # combined input row width = 3104
NSLOT = HPC * 16           # 128 QK slots per core (16 per head)
GW = 130                   # stored group width: 129 data + 1 pad
OCW = 8 * GW               # 1040 output cols per head row

TRACE = False
TRACE_KW: dict = {}
LAST_RESULTS = None

_NC = None


def _build_bass():
    import concourse.mybir as mybir
    import concourse.tile as tile
    from concourse import bacc
    from concourse.tile import add_dep_helper

    f32 = mybir.dt.float32
    DT = mybir.dt.bfloat16
    EXP = mybir.ActivationFunctionType.Exp

    nc = bacc.Bacc("TRN2", target_bir_lowering=False, enable_partition_id=False)
    IN = nc.dram_tensor("inp", [HPC, 128, INW], DT, kind="ExternalInput")
    OC = nc.dram_tensor("oc", [HPC, 128, OCW], DT, kind="ExternalOutput")

    with tile.TileContext(nc) as tc:
        with (
            tc.tile_pool(name="io", bufs=4) as io_pool,
            tc.tile_pool(name="exg", bufs=1) as ex_pool,
            tc.tile_pool(name="outs", bufs=2) as out_pool,
            tc.tile_pool(name="stat", bufs=2) as stat_pool,
            tc.tile_pool(name="spsum", bufs=1, space="PSUM") as s_psum,
            tc.tile_pool(name="vpsum", bufs=2, space="PSUM") as v_psum,
        ):
            # Trigger the ~2.7us exp table-load immediately so it finishes
            # during the DMA ramp.
            warm = stat_pool.tile([128, 1], f32, tag="warm")
            nc.gpsimd.memset(warm[:], 0.0)
            nc.scalar.activation(warm[:], warm[:], EXP)

            sA = s_psum.tile([128, 3 * 512], f32, tag="ringA")
            sB = s_psum.tile([128, 3 * 512], f32, tag="ringB")
            EXG = ex_pool.tile([128, NSLOT * 512], DT, tag="ex")
            pva = v_psum.tile([128, 512], f32, tag="pv", name="pva")
            pvb = v_psum.tile([128, 512], f32, tag="pv", name="pvb")

            ins = [None] * HPC
            outts = [None] * HPC
            load_dmas = {}

            def emit_load(h):
                it = io_pool.tile([128, INW], DT, tag="in", name=f"in_{h}")
                if h == 0:
                    # Head 0 is the ramp critical path: split so QK slots
                    # unlock progressively (slot m needs qt half + kt tile
                    # j=m%8 only), issuing on three parallel queues.  A
                    # 2-byte dummy transfer leads each queue to absorb the
                    # ~2.7us HWDGE cold-start latency; the real transfers
                    # behind it are pipelined and take no dep on it.
                    dwarm = stat_pool.tile([128, 4], DT, tag="dwarm")
                    nc.sync.dma_start(dwarm[:, 0:2], IN[0][:, 0:2])
                    nc.scalar.dma_start(dwarm[:, 2:4], IN[0][:, 2:4])
                    nc.sync.dma_start(it[:, 0:512], IN[0][:, 0:512])
                    nc.scalar.dma_start(it[:, S:S + 128], IN[0][:, S:S + 128])
                    nc.sync.dma_start(it[:, S + 128:S + 384],
                                      IN[0][:, S + 128:S + 384])
                    nc.sync.dma_start(it[:, 512:S], IN[0][:, 512:S])
                    d_kr = nc.scalar.dma_start(it[:, S + 384:2 * S],
                                               IN[0][:, S + 384:2 * S])
                    d_vc = nc.sync.dma_start(it[:, 2 * S:], IN[0][:, 2 * S:])
                    add_dep_helper(d_vc.ins, d_kr.ins, sync=True,
                                   reason="vc0 after ramp-critical kt")
                    load_dmas[0] = d_vc
                else:
                    # Chain prefetch serially behind the previous head so
                    # steady-state loads never crowd ramp-critical HBM
                    # bandwidth (DMA needs ~2.2us/head vs 8.3us period).
                    d_qk = nc.sync.dma_start(it[:, 0:2 * S], IN[h][:, 0:2 * S])
                    d_vc = nc.sync.dma_start(it[:, 2 * S:], IN[h][:, 2 * S:])
                    add_dep_helper(d_qk.ins, load_dmas[h - 1].ins, sync=True,
                                   reason="serialize head prefetch")
                    load_dmas[h] = d_vc
                ins[h] = it

            def vc_j(h, j):
                off = 2 * S + j * VCW
                return ins[h][:, off:off + 129]

            # slot m: head h = m//16, r = m%16, c = r//8 (query half),
            #         j = r%8 (key tile)
            def emit_qk_slot(m):
                h, r = divmod(m, 16)
                c, j = divmod(r, 8)
                it = ins[h]
                st = sA if (m // 3) % 2 == 0 else sB
                nc.tensor.matmul(
                    st[:, (m % 3) * 512:(m % 3 + 1) * 512],
                    it[:, S + j * 128:S + (j + 1) * 128],
                    it[:, c * 512:(c + 1) * 512],
                    start=True, stop=True,
                )

            def emit_act(ms, ln):
                st = sA if (ms // 3) % 2 == 0 else sB
                r = ms % 3
                nc.scalar.activation(
                    EXG[:, ms * 512:(ms + ln) * 512],
                    st[:, r * 512:(r + ln) * 512], EXP, scale=SCALE)

            # PV: per (head, query-half) 4 groups of 128 s-rows.
            # Groups are assigned to 6 rolling psum slices (3 per bank at
            # col offsets 0/130/260), ALTERNATING banks so a group's
            # evacuation cast (DVE read) always overlaps the next group's
            # matmuls in the OTHER bank (same-bank PE-W + DVE-R is fatal
            # and gets serialized by Tile).  Reuse distance = 6 groups.
            # Each half is a list of closures (32 matmuls + casts + store)
            # drained ~CHUNK matmuls per ACT call, so the PE queue
            # alternates [3 QK][~13 PV] and neither engine starves.
            def pv_slice(gidx):
                sg = gidx % 6
                bank = pva if sg % 2 == 0 else pvb
                return bank, 130 * (sg // 2)

            def pv_half_units(h, c):
                if c == 0:
                    outts[h] = out_pool.tile([128, OCW], DT, tag="out",
                                             name=f"out_{h}")
                outt = outts[h]
                base = h * 16 + c * 8
                units = []
                for i in range(4):
                    bank, off = pv_slice(h * 8 + c * 4 + i)
                    ps = bank[:, off:off + 129]
                    for j in range(ST):
                        def mm(ps=ps, h=h, j=j, i=i, base=base):
                            nc.tensor.matmul(
                                ps,
                                EXG[:, (base + j) * 512 + i * 128:(base + j) * 512 + (i + 1) * 128],
                                vc_j(h, j),
                                start=(j == 0), stop=(j == ST - 1),
                            )
                        units.append(("mm", mm))

                    # per-group evacuation cast (to bf16), pad col included
                    g = c * 4 + i

                    def cast(g=g, bank=bank, off=off, outt=outt):
                        nc.vector.tensor_copy(
                            outt[:, g * GW:(g + 1) * GW], bank[:, off:off + GW])
                    units.append(("cast", cast))

                def store(h=h, c=c, outt=outt):
                    nc.sync.dma_start(OC[h][:, c * 520:(c + 1) * 520],
                                      outt[:, c * 520:(c + 1) * 520])
                units.append(("store", store))
                return units

            # Final half (head 7, c=1): j-outer "chase" with the 4 groups
            # in 4 DISTINCT banks (pva, pvb + ring banks 2,3, idle by
            # then), so only the j6/j7 waves trail the last ACTIVATE.
            # Distinct banks are required: interleaved accumulation groups
            # must not share a bank (start=True clears its has_written).
            def pv_chase_units():
                h = HPC - 1
                base = h * 16 + 8
                ps4 = [pva[:, 260:389], pvb[:, 260:389],
                       sB[:, 0:129], sB[:, 512:641]]
                units = []
                for j in range(ST):
                    for i in range(4):
                        def mm(j=j, i=i):
                            nc.tensor.matmul(
                                ps4[i],
                                EXG[:, (base + j) * 512 + i * 128:(base + j) * 512 + (i + 1) * 128],
                                vc_j(h, j),
                                start=(j == 0), stop=(j == ST - 1),
                            )
                        units.append(("mm", mm))

                def evac():
                    outt = outts[h]
                    nc.vector.tensor_copy(outt[:, 520:650], pva[:, 260:390])
                    nc.vector.tensor_copy(outt[:, 650:780], pvb[:, 260:390])
                    nc.scalar.copy(outt[:, 780:910], sB[:, 0:130])
                    nc.scalar.copy(outt[:, 910:1040], sB[:, 512:642])
                    nc.sync.dma_start(OC[h][:, 520:1040], outt[:, 520:1040])
                units.append(("evac", evac))
                return units

            # --- emission ------------------------------------------------
            emit_load(0)
            emit_load(1)
            # A half is drained only once coverage is LAG slots past its
            # last input, so PV matmuls never wait on a just-issued
            # ACTIVATE (zero-lag PV was measured to bubble ~1us per call).
            LAG = 6
            CHUNK = 13
            pv_pending = []
            for h in range(HPC):
                for c in range(2):
                    basec = h * 16 + (c + 1) * 8
                    if (h, c) == (HPC - 1, 1):
                        continue  # final half -> chase, appended at the end
                    thr = basec + LAG if basec + LAG <= NSLOT - 8 else basec
                    pv_pending.append((thr, h, c))
            work_q = []
            covered = 0

            def drain_pv(budget):
                while pv_pending and pv_pending[0][0] <= covered:
                    _, h, c = pv_pending.pop(0)
                    work_q.extend(pv_half_units(h, c))
                while work_q and budget > 0:
                    kind, fn = work_q.pop(0)
                    fn()
                    if kind == "mm":
                        budget -= 1

            m = 0
            pend = 0
            nact = 0
            for h in range(HPC):
                if h + 2 < HPC:
                    emit_load(h + 2)
                for r in range(16):
                    emit_qk_slot(m)
                    m += 1
                    pend += 1
                    # call boundaries: slots {0}, {1,2}, then triples, and a
                    # final pair -- ring triples stay aligned to banks 0-2 /
                    # 3-5.
                    if (nact == 0 and pend == 1) or (nact == 1 and pend == 2) \
                            or pend == 3 or (m == NSLOT and pend > 0):
                        emit_act(m - pend, pend)
                        covered = m
                        pend = 0
                        nact += 1
                        # skip the drain right before the final pair so QK
                        # slots 126/127 aren't FIFO-delayed behind PV work;
                        # the deferred chunk runs under the final ACTIVATE.
                        if m < NSLOT - 2:
                            drain_pv(CHUNK)
            work_q.extend(pv_chase_units())
            drain_pv(10 ** 9)

    nc.compile()
    return nc


def _get_nc():
    global _NC
    if _NC is None:
        _NC = _build_bass()
    return _NC


def kernel(Qx, Kx, Vx, Qy, Ky, Vy):
    global LAST_RESULTS
    bf = ml_dtypes.bfloat16
    Qx, Kx, Vx, Qy, Ky, Vy = (
        np.asarray(t, dtype=np.float32) for t in (Qx, Kx, Vx, Qy, Ky, Vy)
    )

    qx = Qx.reshape(HEADS, S, D)
    qy = Qy.reshape(HEADS, S, D)
    kx = Kx.reshape(HEADS, S, D)
    ky = Ky.reshape(HEADS, S, D)
    vx = Vx.reshape(HEADS, S, D)
    vy = Vy.reshape(HEADS, S, D)

    # Combined per-head input block: [head, p=128, INW] where
    #   [:, 0:S]        = QT (x stream on partitions 0:64, y on 64:128)
    #   [:, S:2S]       = KT (same partition split)
    #   [:, 2S + j*VCW + c] = VC: kv position t = j*128+p; c in [Vx|Vy|1|pad]
    IN = np.zeros((HEADS, 128, INW), np.float32)
    IN[:, :D, 0:S] = qx.transpose(0, 2, 1)
    IN[:, D:, 0:S] = qy.transpose(0, 2, 1)
    IN[:, :D, S:2 * S] = kx.transpose(0, 2, 1)
    IN[:, D:, S:2 * S] = ky.transpose(0, 2, 1)
    vc = IN[:, :, 2 * S:].reshape(HEADS, 128, ST, VCW)
    vc[..., :D] = vx.reshape(HEADS, ST, 128, D).transpose(0, 2, 1, 3)
    vc[..., D:2 * D] = vy.reshape(HEADS, ST, 128, D).transpose(0, 2, 1, 3)
    vc[..., 2 * D] = 1.0

    in_maps = []
    for c in range(N_CORES):
        sl = slice(c * HPC, (c + 1) * HPC)
        in_maps.append({"inp": IN[sl].astype(bf)})

    from concourse.bass_utils import run_bass_kernel_spmd

    nc = _get_nc()
    res = run_bass_kernel_spmd(
        nc, in_maps, core_ids=list(range(N_CORES)), trace=TRACE, **TRACE_KW
    )
    LAST_RESULTS = res

    # oc: per core [HPC, p=128, OCW] bf16, 8 groups of GW=130 cols:
    # group g (= s-block) has [out1_raw(64) | out2_raw(64) | sumexp | pad];
    # row s = g*128 + p.  softmax normalization happens here on host.
    oc = np.concatenate([np.asarray(r["oc"]) for r in res.results], axis=0)
    oc = oc.astype(np.float32).reshape(HEADS, 128, ST, GW)
    oc = oc.transpose(0, 2, 1, 3).reshape(B, H, S, GW)
    z = oc[..., 2 * D:2 * D + 1]
    out1 = np.ascontiguousarray(oc[..., :D] / z)
    out2 = np.ascontiguousarray(oc[..., D:2 * D] / z)
    return out1, out2


# revision 30
# speedup vs baseline: 1.0146x; 1.0146x over previous
# Trainium2 Bass kernel for nn_CalculateAttention_7722351198463
#
# reference computes, per (batch, head):
#   scores = (Qx @ Kx^T + Qy @ Ky^T) * 0.5 / sqrt(D)
#   attn   = softmax(scores, axis=-1)
#   out1   = attn @ Vx ; out2 = attn @ Vy
#
# Sharding: B*H = 64 heads, 8 heads per core across 8 NeuronCores (no comms).
#
# Device-side design (per core, 8 heads), v2 (87.4us baseline -> 82.8us):
#  * Host pre-transposes Q/K into QT/KT = [d=128, s=1024] per head (x stream
#    on partitions 0:64, y on 64:128); one bf16 matmul with contraction 128
#    computes Qx@Kx^T + Qy@Ky^T in one pass, in transposed [t, s] layout.
#  * The core's QK output is a single global stream of 128 "slots" (one slot
#    = one N=512 matmul = one PSUM bank).  Slots ping-pong between TWO
#    3-bank PSUM tiles sA/sB; ScalarE exp() consumes a whole tile per
#    ACTIVATE (N=1536), cutting the ~260ns per-call ACT overhead 3x vs
#    per-1024 calls.  The exp stream is the kernel bottleneck (ScalarE =
#    128 lanes @ 1.2GHz: 54.6us + 44 x 0.26us overhead ~= 66us busy) and
#    runs gapless in steady state.  sA/sB MUST be separate tiles: PSUM dep
#    tracking conflates slices within one tile, which serialized QK behind
#    the previous ACTIVATE (+950ns/call).
#  * exp lands in one persistent write-once SBUF tensor EX[128, 128*512]
#    (bf16), so ACTIVATE calls may cross head boundaries.
#  * QK slots are c-major within a head (all 8 key-tiles for query columns
#    0:512, then 512:1024), so each PV group's accumulation spans ~2.7 ACT
#    calls and at most ~4 of 8 groups per head are PSUM-resident, fitting 2
#    PV banks (3 slices/bank at col offsets 0/130/260).  Slices alternate
#    banks group-to-group so evacuation casts (DVE) never share a bank with
#    live PV matmuls (same-bank PE-W + DVE-R is fatal -> Tile serializes).
#  * PV work is emitted as closures drained ~13 matmuls per ACT call with a
#    6-slot lag, so the PE queue alternates [3 QK][~13 PV]; a dep-blocked
#    PV matmul can never starve ACT of its next QK slots (strict PE FIFO).
#  * V is packed host-side as VC = [t, 132] = [Vx | Vy | ones | pad]: one
#    accumulating matmul chain per (head, s-block) computes
#    [out1_raw | out2_raw | sumexp]; normalization happens on host.
#  * Outputs are stored bf16 (raw + sumexp), halving output DMA.
#  * Ramp: exp-table load triggered immediately via a zero-dep warm
#    activation; head 0's DMA is split per-matmul so the first (mini)
#    ACTIVATE waits only on qt[:,0:512] + kt tile 0; later prefetch is
#    chained serially off the ramp-critical path.
#  * Tail: the final half runs j-outer across 4 distinct banks (pva, pvb +
#    2 idle sB banks) so only its j6/j7 waves trail the last ACTIVATE.
import numpy as np
import ml_dtypes

B, H, S, D = 4, 16, 1024, 64
N_CORES = 8
HEADS = B * H              # 64
HPC = HEADS // N_CORES     # heads per core = 8
ST = S // 128              # key tiles per head = 8
SCALE = 0.5 / 8.0          # 0.5 / sqrt(D)
VCW = 132                  # packed V width: 64 + 64 + 1 (ones) + 3 pad
INW = 2 * S + ST * VCW     # combined input row width = 3104
NSLOT = HPC * 16           # 128 QK slots per core (16 per head)
GW = 130                   # stored group width: 129 data + 1 pad
OCW = 8 * GW               # 1040 output cols per head row

TRACE = False
TRACE_KW: dict = {}
LAST_RESULTS = None

_NC = None


def _build_bass():
    import concourse.mybir as mybir
    import concourse.tile as tile
    from concourse import bacc
    from concourse.tile import add_dep_helper

    f32 = mybir.dt.float32
    DT = mybir.dt.bfloat16
    EXP = mybir.ActivationFunctionType.Exp

    nc = bacc.Bacc("TRN2", target_bir_lowering=False, enable_partition_id=False)
    IN = nc.dram_tensor("inp", [HPC, 128, INW], DT, kind="ExternalInput")
    OC = nc.dram_tensor("oc", [HPC, 128, OCW], DT, kind="ExternalOutput")

    with tile.TileContext(nc) as tc:
        with (
            tc.tile_pool(name="io", bufs=4) as io_pool,
            tc.tile_pool(name="exg", bufs=1) as ex_pool,
            tc.tile_pool(name="outs", bufs=2) as out_pool,
            tc.tile_pool(name="stat", bufs=2) as stat_pool,
            tc.tile_pool(name="spsum", bufs=1, space="PSUM") as s_psum,
            tc.tile_pool(name="vpsum", bufs=2, space="PSUM") as v_psum,
        ):
            # Trigger the ~2.7us exp table-load immediately so it finishes
            # during the DMA ramp.
            warm = stat_pool.tile([128, 1], f32, tag="warm")
            nc.gpsimd.memset(warm[:], 0.0)
            nc.scalar.activation(warm[:], warm[:], EXP)

            sA = s_psum.tile([128, 3 * 512], f32, tag="ringA")
            sB = s_psum.tile([128, 3 * 512], f32, tag="ringB")
            EXG = ex_pool.tile([128, NSLOT * 512], DT, tag="ex")
            pva = v_psum.tile([128, 512], f32, tag="pv", name="pva")
            pvb = v_psum.tile([128, 512], f32, tag="pv", name="pvb")

            ins = [None] * HPC
            outts = [None] * HPC
            load_dmas = {}

            def emit_load(h):
                it = io_pool.tile([128, INW], DT, tag="in", name=f"in_{h}")
                if h == 0:
                    # Head 0 is the ramp critical path: split so QK slots
                    # unlock progressively (slot m needs qt half + kt tile
                    # j=m%8 only), issuing on three parallel queues.
                    nc.sync.dma_start(it[:, 0:512], IN[0][:, 0:512])
                    nc.scalar.dma_start(it[:, S:S + 128], IN[0][:, S:S + 128])
                    nc.sync.dma_start(it[:, S + 128:S + 384],
                                      IN[0][:, S + 128:S + 384])
                    nc.sync.dma_start(it[:, 512:S], IN[0][:, 512:S])
                    d_kr = nc.scalar.dma_start(it[:, S + 384:2 * S],
                                               IN[0][:, S + 384:2 * S])
                    d_vc = nc.sync.dma_start(it[:, 2 * S:], IN[0][:, 2 * S:])
                    add_dep_helper(d_vc.ins, d_kr.ins, sync=True,
                                   reason="vc0 after ramp-critical kt")
                    load_dmas[0] = d_vc
                else:
                    # Chain prefetch serially behind the previous head so
                    # steady-state loads never crowd ramp-critical HBM
                    # bandwidth (DMA needs ~2.2us/head vs 8.3us period).
                    d_qk = nc.sync.dma_start(it[:, 0:2 * S], IN[h][:, 0:2 * S])
                    d_vc = nc.sync.dma_start(it[:, 2 * S:], IN[h][:, 2 * S:])
                    add_dep_helper(d_qk.ins, load_dmas[h - 1].ins, sync=True,
                                   reason="serialize head prefetch")
                    load_dmas[h] = d_vc
                ins[h] = it

            def vc_j(h, j):
                off = 2 * S + j * VCW
                return ins[h][:, off:off + 129]

            # slot m: head h = m//16, r = m%16, c = r//8 (query half),
            #         j = r%8 (key tile)
            def emit_qk_slot(m):
                h, r = divmod(m, 16)
                c, j = divmod(r, 8)
                it = ins[h]
                st = sA if (m // 3) % 2 == 0 else sB
                nc.tensor.matmul(
                    st[:, (m % 3) * 512:(m % 3 + 1) * 512],
                    it[:, S + j * 128:S + (j + 1) * 128],
                    it[:, c * 512:(c + 1) * 512],
                    start=True, stop=True,
                )

            def emit_act(ms, ln):
                st = sA if (ms // 3) % 2 == 0 else sB
                r = ms % 3
                nc.scalar.activation(
                    EXG[:, ms * 512:(ms + ln) * 512],
                    st[:, r * 512:(r + ln) * 512], EXP, scale=SCALE)

            # PV: per (head, query-half) 4 groups of 128 s-rows.
            # Groups are assigned to 6 rolling psum slices (3 per bank at
            # col offsets 0/130/260), ALTERNATING banks so a group's
            # evacuation cast (DVE read) always overlaps the next group's
            # matmuls in the OTHER bank (same-bank PE-W + DVE-R is fatal
            # and gets serialized by Tile).  Reuse distance = 6 groups.
            # Each half is a list of closures (32 matmuls + casts + store)
            # drained ~CHUNK matmuls per ACT call, so the PE queue
            # alternates [3 QK][~13 PV] and neither engine starves.
            def pv_slice(gidx):
                sg = gidx % 6
                bank = pva if sg % 2 == 0 else pvb
                return bank, 130 * (sg // 2)

            def pv_half_units(h, c):
                if c == 0:
                    outts[h] = out_pool.tile([128, OCW], DT, tag="out",
                                             name=f"out_{h}")
                outt = outts[h]
                base = h * 16 + c * 8
                units = []
                for i in range(4):
                    bank, off = pv_slice(h * 8 + c * 4 + i)
                    ps = bank[:, off:off + 129]
                    for j in range(ST):
                        def mm(ps=ps, h=h, j=j, i=i, base=base):
                            nc.tensor.matmul(
                                ps,
                                EXG[:, (base + j) * 512 + i * 128:(base + j) * 512 + (i + 1) * 128],
                                vc_j(h, j),
                                start=(j == 0), stop=(j == ST - 1),
                            )
                        units.append(("mm", mm))

                    # per-group evacuation cast (to bf16), pad col included
                    g = c * 4 + i

                    def cast(g=g, bank=bank, off=off, outt=outt):
                        nc.vector.tensor_copy(
                            outt[:, g * GW:(g + 1) * GW], bank[:, off:off + GW])
                    units.append(("cast", cast))

                def store(h=h, c=c, outt=outt):
                    nc.sync.dma_start(OC[h][:, c * 520:(c + 1) * 520],
                                      outt[:, c * 520:(c + 1) * 520])
                units.append(("store", store))
                return units

            # Final half (head 7, c=1): j-outer "chase" with the 4 groups
            # in 4 DISTINCT banks (pva, pvb + ring banks 2,3, idle by
            # then), so only the j6/j7 waves trail the last ACTIVATE.
            # Distinct banks are required: interleaved accumulation groups
            # must not share a bank (start=True clears its has_written).
            def pv_chase_units():
                h = HPC - 1
                base = h * 16 + 8
                ps4 = [pva[:, 260:389], pvb[:, 260:389],
                       sB[:, 0:129], sB[:, 512:641]]
                units = []
                for j in range(ST):
                    for i in range(4):
                        def mm(j=j, i=i):
                            nc.tensor.matmul(
                                ps4[i],
                                EXG[:, (base + j) * 512 + i * 128:(base + j) * 512 + (i + 1) * 128],
                                vc_j(h, j),
                                start=(j == 0), stop=(j == ST - 1),
                            )
                        units.append(("mm", mm))

                def evac():
                    outt = outts[h]
                    nc.vector.tensor_copy(outt[:, 520:650], pva[:, 260:390])
                    nc.vector.tensor_copy(outt[:, 650:780], pvb[:, 260:390])
                    nc.scalar.copy(outt[:, 780:910], sB[:, 0:130])
                    nc.scalar.copy(outt[:, 910:1040], sB[:, 512:642])
                    nc.sync.dma_start(OC[h][:, 520:1040], outt[:, 520:1040])
                units.append(("evac", evac))
                return units

            # --- emission ------------------------------------------------
            emit_load(0)
            emit_load(1)
            # A half is drained only once coverage is LAG slots past its
            # last input, so PV matmuls never wait on a just-issued
            # ACTIVATE (zero-lag PV was measured to bubble ~1us per call).
            LAG = 6
            CHUNK = 13
            pv_pending = []
            for h in range(HPC):
                for c in range(2):
                    basec = h * 16 + (c + 1) * 8
                    if (h, c) == (HPC - 1, 1):
                        continue  # final half -> chase, appended at the end
                    thr = basec + LAG if basec + LAG <= NSLOT - 8 else basec
                    pv_pending.append((thr, h, c))
            work_q = []
            covered = 0

            def drain_pv(budget):
                while pv_pending and pv_pending[0][0] <= covered:
                    _, h, c = pv_pending.pop(0)
                    work_q.extend(pv_half_units(h, c))
                while work_q and budget > 0:
                    kind, fn = work_q.pop(0)
                    fn()
                    if kind == "mm":
                        budget -= 1

            m = 0
            pend = 0
            nact = 0
            for h in range(HPC):
                if h + 2 < HPC:
                    emit_load(h + 2)
                for r in range(16):
                    emit_qk_slot(m)
                    m += 1
                    pend += 1
                    # call boundaries: slots {0}, {1,2}, then triples, and a
                    # final pair -- ring triples stay aligned to banks 0-2 /
                    # 3-5.
                    if (nact == 0 and pend == 1) or (nact == 1 and pend == 2) \
                            or pend == 3 or (m == NSLOT and pend > 0):
                        emit_act(m - pend, pend)
                        covered = m
                        pend = 0
                        nact += 1
                        # skip the drain right before the final pair so QK
                        # slots 126/127 aren't FIFO-delayed behind PV work;
                        # the deferred chunk runs under the final ACTIVATE.
                        if m < NSLOT - 2:
                            drain_pv(CHUNK)
            work_q.extend(pv_chase_units())
            drain_pv(10 ** 9)

    nc.compile()
    return nc


def _get_nc():
    global _NC
    if _NC is None:
        _NC = _build_bass()
    return _NC


def kernel(Qx, Kx, Vx, Qy, Ky, Vy):
    global LAST_RESULTS
    bf = ml_dtypes.bfloat16
    Qx, Kx, Vx, Qy, Ky, Vy = (
        np.asarray(t, dtype=np.float32) for t in (Qx, Kx, Vx, Qy, Ky, Vy)
    )

    qx = Qx.reshape(HEADS, S, D)
    qy = Qy.reshape(HEADS, S, D)
    kx = Kx.reshape(HEADS, S, D)
    ky = Ky.reshape(HEADS, S, D)
    vx = Vx.reshape(HEADS, S, D)
    vy = Vy.reshape(HEADS, S, D)

    # Combined per-head input block: [head, p=128, INW] where
    #   [:, 0:S]        = QT (x stream on partitions 0:64, y on 64:128)
    #   [:, S:2S]       = KT (same partition split)
    #   [:, 2S + j*VCW + c] = VC: kv position t = j*128+p; c in [Vx|Vy|1|pad]
    IN = np.zeros((HEADS, 128, INW), np.float32)
    IN[:, :D, 0:S] = qx.transpose(0, 2, 1)
    IN[:, D:, 0:S] = qy.transpose(0, 2, 1)
    IN[:, :D, S:2 * S] = kx.transpose(0, 2, 1)
    IN[:, D:, S:2 * S] = ky.transpose(0, 2, 1)
    vc = IN[:, :, 2 * S:].reshape(HEADS, 128, ST, VCW)
    vc[..., :D] = vx.reshape(HEADS, ST, 128, D).transpose(0, 2, 1, 3)
    vc[..., D:2 * D] = vy.reshape(HEADS, ST, 128, D).transpose(0, 2, 1, 3)
    vc[..., 2 * D] = 1.0

    in_maps = []
    for c in range(N_CORES):
        sl = slice(c * HPC, (c + 1) * HPC)
        in_maps.append({"inp": IN[sl].astype(bf)})

    from concourse.bass_utils import run_bass_kernel_spmd

    nc = _get_nc()
    res = run_bass_kernel_spmd(
        nc, in_maps, core_ids=list(range(N_CORES)), trace=TRACE, **TRACE_KW
    )
    LAST_RESULTS = res

    # oc: per core [HPC, p=128, OCW] bf16, 8 groups of GW=130 cols:
    # group g (= s-block) has [out1_raw(64) | out2_raw(64) | sumexp | pad];
    # row s = g*128 + p.  softmax normalization happens here on host.
    oc = np.concatenate([np.asarray(r["oc"]) for r in res.results], axis=0)
    oc = oc.astype(np.float32).reshape(HEADS, 128, ST, GW)
    oc = oc.transpose(0, 2, 1, 3).reshape(B, H, S, GW)
    z = oc[..., 2 * D:2 * D + 1]
    out1 = np.ascontiguousarray(oc[..., :D] / z)
    out2 = np.ascontiguousarray(oc[..., D:2 * D] / z)
    return out1, out2


# revision 31
# speedup vs baseline: 1.0174x; 1.0027x over previous
# Trainium2 Bass kernel for nn_CalculateAttention_7722351198463
#
# reference computes, per (batch, head):
#   scores = (Qx @ Kx^T + Qy @ Ky^T) * 0.5 / sqrt(D)
#   attn   = softmax(scores, axis=-1)
#   out1   = attn @ Vx ; out2 = attn @ Vy
#
# Sharding: B*H = 64 heads, 8 heads per core across 8 NeuronCores (no comms).
#
# Device-side design (per core, 8 heads), v2 (87.4us baseline -> 82.8us):
#  * Host pre-transposes Q/K into QT/KT = [d=128, s=1024] per head (x stream
#    on partitions 0:64, y on 64:128); one bf16 matmul with contraction 128
#    computes Qx@Kx^T + Qy@Ky^T in one pass, in transposed [t, s] layout.
#  * The core's QK output is a single global stream of 128 "slots" (one slot
#    = one N=512 matmul = one PSUM bank).  Slots ping-pong between TWO
#    3-bank PSUM tiles sA/sB; ScalarE exp() consumes a whole tile per
#    ACTIVATE (N=1536), cutting the ~260ns per-call ACT overhead 3x vs
#    per-1024 calls.  The exp stream is the kernel bottleneck (ScalarE =
#    128 lanes @ 1.2GHz: 54.6us + 44 x 0.26us overhead ~= 66us busy) and
#    runs gapless in steady state.  sA/sB MUST be separate tiles: PSUM dep
#    tracking conflates slices within one tile, which serialized QK behind
#    the previous ACTIVATE (+950ns/call).
#  * exp lands in one persistent write-once SBUF tensor EX[128, 128*512]
#    (bf16), so ACTIVATE calls may cross head boundaries.
#  * QK slots are c-major within a head (all 8 key-tiles for query columns
#    0:512, then 512:1024), so each PV group's accumulation spans ~2.7 ACT
#    calls and at most ~4 of 8 groups per head are PSUM-resident, fitting 2
#    PV banks (3 slices/bank at col offsets 0/130/260).  Slices alternate
#    banks group-to-group so evacuation casts (DVE) never share a bank with
#    live PV matmuls (same-bank PE-W + DVE-R is fatal -> Tile serializes).
#  * PV work is emitted as closures drained ~13 matmuls per ACT call with a
#    6-slot lag, so the PE queue alternates [3 QK][~13 PV]; a dep-blocked
#    PV matmul can never starve ACT of its next QK slots (strict PE FIFO).
#  * V is packed host-side as VC = [t, 132] = [Vx | Vy | ones | pad]: one
#    accumulating matmul chain per (head, s-block) computes
#    [out1_raw | out2_raw | sumexp]; normalization happens on host.
#  * Outputs are stored bf16 (raw + sumexp), halving output DMA.
#  * Ramp: exp-table load triggered immediately via a zero-dep warm
#    activation; head 0's DMA is split per-matmul so the first (mini)
#    ACTIVATE waits only on qt[:,0:512] + kt tile 0; later prefetch is
#    chained serially off the ramp-critical path.
#  * Tail: the final half runs j-outer across 4 distinct banks (pva, pvb +
#    2 idle sB banks) so only its j6/j7 waves trail the last ACTIVATE.
import numpy as np
import ml_dtypes

B, H, S, D = 4, 16, 1024, 64
N_CORES = 8
HEADS = B * H              # 64
HPC = HEADS // N_CORES     # heads per core = 8
ST = S // 128              # key tiles per head = 8
SCALE = 0.5 / 8.0          # 0.5 / sqrt(D)
VCW = 132                  # packed V width: 64 + 64 + 1 (ones) + 3 pad
INW = 2 * S + ST * VCW     # combined input row width = 3104
NSLOT = HPC * 16           # 128 QK slots per core (16 per head)
GW = 130                   # stored group width: 129 data + 1 pad
OCW = 8 * GW               # 1040 output cols per head row

TRACE = False
TRACE_KW: dict = {}
LAST_RESULTS = None

_NC = None


def _build_bass():
    import concourse.mybir as mybir
    import concourse.tile as tile
    from concourse import bacc
    from concourse.tile import add_dep_helper

    f32 = mybir.dt.float32
    DT = mybir.dt.bfloat16
    EXP = mybir.ActivationFunctionType.Exp

    nc = bacc.Bacc("TRN2", target_bir_lowering=False, enable_partition_id=False)
    IN = nc.dram_tensor("inp", [HPC, 128, INW], DT, kind="ExternalInput")
    OC = nc.dram_tensor("oc", [HPC, 128, OCW], DT, kind="ExternalOutput")

    with tile.TileContext(nc) as tc:
        with (
            tc.tile_pool(name="io", bufs=4) as io_pool,
            tc.tile_pool(name="exg", bufs=1) as ex_pool,
            tc.tile_pool(name="outs", bufs=2) as out_pool,
            tc.tile_pool(name="stat", bufs=2) as stat_pool,
            tc.tile_pool(name="spsum", bufs=1, space="PSUM") as s_psum,
            tc.tile_pool(name="vpsum", bufs=2, space="PSUM") as v_psum,
        ):
            # Trigger the ~2.7us exp table-load immediately so it finishes
            # during the DMA ramp.
            warm = stat_pool.tile([128, 1], f32, tag="warm")
            nc.gpsimd.memset(warm[:], 0.0)
            nc.scalar.activation(warm[:], warm[:], EXP)

            sA = s_psum.tile([128, 3 * 512], f32, tag="ringA")
            sB = s_psum.tile([128, 3 * 512], f32, tag="ringB")
            EXG = ex_pool.tile([128, NSLOT * 512], DT, tag="ex")
            pva = v_psum.tile([128, 512], f32, tag="pv", name="pva")
            pvb = v_psum.tile([128, 512], f32, tag="pv", name="pvb")

            ins = [None] * HPC
            outts = [None] * HPC
            load_dmas = {}

            def emit_load(h):
                it = io_pool.tile([128, INW], DT, tag="in", name=f"in_{h}")
                if h == 0:
                    # Head 0 is the ramp critical path: split so QK slots
                    # unlock progressively (slot m needs qt half + kt tile
                    # j=m%8 only), issuing on three parallel queues.
                    nc.sync.dma_start(it[:, 0:512], IN[0][:, 0:512])
                    nc.scalar.dma_start(it[:, S:S + 128], IN[0][:, S:S + 128])
                    nc.sync.dma_start(it[:, S + 128:S + 384],
                                      IN[0][:, S + 128:S + 384])
                    nc.sync.dma_start(it[:, 512:S], IN[0][:, 512:S])
                    d_kr = nc.scalar.dma_start(it[:, S + 384:2 * S],
                                               IN[0][:, S + 384:2 * S])
                    d_vc = nc.sync.dma_start(it[:, 2 * S:], IN[0][:, 2 * S:])
                    add_dep_helper(d_vc.ins, d_kr.ins, sync=True,
                                   reason="vc0 after ramp-critical kt")
                    # head 1 may overlap vc0 (the ramp-critical kt is done
                    # by then), so anchor the chain on kt-rest, not vc0.
                    load_dmas[0] = d_kr
                else:
                    # Chain prefetch serially behind the previous head so
                    # steady-state loads never crowd ramp-critical HBM
                    # bandwidth (DMA needs ~2.2us/head vs 8.3us period).
                    d_qk = nc.sync.dma_start(it[:, 0:2 * S], IN[h][:, 0:2 * S])
                    d_vc = nc.sync.dma_start(it[:, 2 * S:], IN[h][:, 2 * S:])
                    add_dep_helper(d_qk.ins, load_dmas[h - 1].ins, sync=True,
                                   reason="serialize head prefetch")
                    load_dmas[h] = d_vc
                ins[h] = it

            def vc_j(h, j):
                off = 2 * S + j * VCW
                return ins[h][:, off:off + 129]

            # slot m: head h = m//16, r = m%16, c = r//8 (query half),
            #         j = r%8 (key tile)
            def emit_qk_slot(m):
                h, r = divmod(m, 16)
                c, j = divmod(r, 8)
                it = ins[h]
                st = sA if (m // 3) % 2 == 0 else sB
                nc.tensor.matmul(
                    st[:, (m % 3) * 512:(m % 3 + 1) * 512],
                    it[:, S + j * 128:S + (j + 1) * 128],
                    it[:, c * 512:(c + 1) * 512],
                    start=True, stop=True,
                )

            def emit_act(ms, ln):
                st = sA if (ms // 3) % 2 == 0 else sB
                r = ms % 3
                nc.scalar.activation(
                    EXG[:, ms * 512:(ms + ln) * 512],
                    st[:, r * 512:(r + ln) * 512], EXP, scale=SCALE)

            # PV: per (head, query-half) 4 groups of 128 s-rows.
            # Groups are assigned to 6 rolling psum slices (3 per bank at
            # col offsets 0/130/260), ALTERNATING banks so a group's
            # evacuation cast (DVE read) always overlaps the next group's
            # matmuls in the OTHER bank (same-bank PE-W + DVE-R is fatal
            # and gets serialized by Tile).  Reuse distance = 6 groups.
            # Each half is a list of closures (32 matmuls + casts + store)
            # drained ~CHUNK matmuls per ACT call, so the PE queue
            # alternates [3 QK][~13 PV] and neither engine starves.
            def pv_slice(gidx):
                sg = gidx % 6
                bank = pva if sg % 2 == 0 else pvb
                return bank, 130 * (sg // 2)

            def pv_half_units(h, c):
                if c == 0:
                    outts[h] = out_pool.tile([128, OCW], DT, tag="out",
                                             name=f"out_{h}")
                outt = outts[h]
                base = h * 16 + c * 8
                units = []
                for i in range(4):
                    bank, off = pv_slice(h * 8 + c * 4 + i)
                    ps = bank[:, off:off + 129]
                    for j in range(ST):
                        def mm(ps=ps, h=h, j=j, i=i, base=base):
                            nc.tensor.matmul(
                                ps,
                                EXG[:, (base + j) * 512 + i * 128:(base + j) * 512 + (i + 1) * 128],
                                vc_j(h, j),
                                start=(j == 0), stop=(j == ST - 1),
                            )
                        units.append(("mm", mm))

                    # per-group evacuation cast (to bf16), pad col included
                    g = c * 4 + i

                    def cast(g=g, bank=bank, off=off, outt=outt):
                        nc.vector.tensor_copy(
                            outt[:, g * GW:(g + 1) * GW], bank[:, off:off + GW])
                    units.append(("cast", cast))

                def store(h=h, c=c, outt=outt):
                    nc.sync.dma_start(OC[h][:, c * 520:(c + 1) * 520],
                                      outt[:, c * 520:(c + 1) * 520])
                units.append(("store", store))
                return units

            # Final half (head 7, c=1): j-outer "chase" with the 4 groups
            # in 4 DISTINCT banks (pva, pvb + ring banks 2,3, idle by
            # then), so only the j6/j7 waves trail the last ACTIVATE.
            # Distinct banks are required: interleaved accumulation groups
            # must not share a bank (start=True clears its has_written).
            def pv_chase_units():
                h = HPC - 1
                base = h * 16 + 8
                ps4 = [pva[:, 260:389], pvb[:, 260:389],
                       sB[:, 0:129], sB[:, 512:641]]
                units = []
                for j in range(ST):
                    for i in range(4):
                        def mm(j=j, i=i):
                            nc.tensor.matmul(
                                ps4[i],
                                EXG[:, (base + j) * 512 + i * 128:(base + j) * 512 + (i + 1) * 128],
                                vc_j(h, j),
                                start=(j == 0), stop=(j == ST - 1),
                            )
                        units.append(("mm", mm))

                def evac():
                    outt = outts[h]
                    nc.vector.tensor_copy(outt[:, 520:650], pva[:, 260:390])
                    nc.vector.tensor_copy(outt[:, 650:780], pvb[:, 260:390])
                    nc.scalar.copy(outt[:, 780:910], sB[:, 0:130])
                    nc.scalar.copy(outt[:, 910:1040], sB[:, 512:642])
                    nc.sync.dma_start(OC[h][:, 520:1040], outt[:, 520:1040])
                units.append(("evac", evac))
                return units

            # --- emission ------------------------------------------------
            emit_load(0)
            emit_load(1)
            # A half is drained only once coverage is LAG slots past its
            # last input, so PV matmuls never wait on a just-issued
            # ACTIVATE (zero-lag PV was measured to bubble ~1us per call).
            LAG = 6
            CHUNK = 13
            pv_pending = []
            for h in range(HPC):
                for c in range(2):
                    basec = h * 16 + (c + 1) * 8
                    if (h, c) == (HPC - 1, 1):
                        continue  # final half -> chase, appended at the end
                    thr = basec + LAG if basec + LAG <= NSLOT - 8 else basec
                    pv_pending.append((thr, h, c))
            work_q = []
            covered = 0

            def drain_pv(budget):
                while pv_pending and pv_pending[0][0] <= covered:
                    _, h, c = pv_pending.pop(0)
                    work_q.extend(pv_half_units(h, c))
                while work_q and budget > 0:
                    kind, fn = work_q.pop(0)
                    fn()
                    if kind == "mm":
                        budget -= 1

            m = 0
            pend = 0
            nact = 0
            for h in range(HPC):
                if h + 2 < HPC:
                    emit_load(h + 2)
                for r in range(16):
                    emit_qk_slot(m)
                    m += 1
                    pend += 1
                    # call boundaries: slots {0}, {1,2}, then triples, and a
                    # final pair -- ring triples stay aligned to banks 0-2 /
                    # 3-5.
                    if (nact == 0 and pend == 1) or (nact == 1 and pend == 2) \
                            or pend == 3 or (m == NSLOT and pend > 0):
                        emit_act(m - pend, pend)
                        covered = m
                        pend = 0
                        nact += 1
                        # skip the drain right before the final pair so QK
                        # slots 126/127 aren't FIFO-delayed behind PV work;
                        # the deferred chunk runs under the final ACTIVATE.
                        if m < NSLOT - 2:
                            drain_pv(CHUNK)
            work_q.extend(pv_chase_units())
            drain_pv(10 ** 9)

    nc.compile()
    return nc


def _get_nc():
    global _NC
    if _NC is None:
        _NC = _build_bass()
    return _NC


def kernel(Qx, Kx, Vx, Qy, Ky, Vy):
    global LAST_RESULTS
    bf = ml_dtypes.bfloat16
    Qx, Kx, Vx, Qy, Ky, Vy = (
        np.asarray(t, dtype=np.float32) for t in (Qx, Kx, Vx, Qy, Ky, Vy)
    )

    qx = Qx.reshape(HEADS, S, D)
    qy = Qy.reshape(HEADS, S, D)
    kx = Kx.reshape(HEADS, S, D)
    ky = Ky.reshape(HEADS, S, D)
    vx = Vx.reshape(HEADS, S, D)
    vy = Vy.reshape(HEADS, S, D)

    # Combined per-head input block: [head, p=128, INW] where
    #   [:, 0:S]        = QT (x stream on partitions 0:64, y on 64:128)
    #   [:, S:2S]       = KT (same partition split)
    #   [:, 2S + j*VCW + c] = VC: kv position t = j*128+p; c in [Vx|Vy|1|pad]
    IN = np.zeros((HEADS, 128, INW), np.float32)
    IN[:, :D, 0:S] = qx.transpose(0, 2, 1)
    IN[:, D:, 0:S] = qy.transpose(0, 2, 1)
    IN[:, :D, S:2 * S] = kx.transpose(0, 2, 1)
    IN[:, D:, S:2 * S] = ky.transpose(0, 2, 1)
    vc = IN[:, :, 2 * S:].reshape(HEADS, 128, ST, VCW)
    vc[..., :D] = vx.reshape(HEADS, ST, 128, D).transpose(0, 2, 1, 3)
    vc[..., D:2 * D] = vy.reshape(HEADS, ST, 128, D).transpose(0, 2, 1, 3)
    vc[..., 2 * D] = 1.0

    in_maps = []
    for c in range(N_CORES):
        sl = slice(c * HPC, (c + 1) * HPC)
        in_maps.append({"inp": IN[sl].astype(bf)})

    from concourse.bass_utils import run_bass_kernel_spmd

    nc = _get_nc()
    res = run_bass_kernel_spmd(
        nc, in_maps, core_ids=list(range(N_CORES)), trace=TRACE, **TRACE_KW
    )
    LAST_RESULTS = res

    # oc: per core [HPC, p=128, OCW] bf16, 8 groups of GW=130 cols:
    # group g (= s-block) has [out1_raw(64) | out2_raw(64) | sumexp | pad];
    # row s = g*128 + p.  softmax normalization happens here on host.
    oc = np.concatenate([np.asarray(r["oc"]) for r in res.results], axis=0)
    oc = oc.astype(np.float32).reshape(HEADS, 128, ST, GW)
    oc = oc.transpose(0, 2, 1, 3).reshape(B, H, S, GW)
    z = oc[..., 2 * D:2 * D + 1]
    out1 = np.ascontiguousarray(oc[..., :D] / z)
    out2 = np.ascontiguousarray(oc[..., D:2 * D] / z)
    return out1, out2
